# revision 35
# baseline (speedup 1.0000x reference)
"""Bahdanau attention kernel for 8x Trainium2 NeuronCores (Bass/Tile).

Problem (per reference):
  B=32, S=2048, EH=DH=512, A=512
  dh    = transpose(decoder_hidden,(1,0,2)).reshape(B, 1024)
  temp1 = enc @ W                  # (B,S,A)
  temp2 = dh @ U                   # (B,A)
  score = tanh(temp1 + temp2) @ v  # (B,S)
  attn  = softmax(score, -1)       # (B,S)
  ctx   = attn @ enc               # (B,1,1024)

Sharding: data-parallel over batch, 4 batches/core. Each core receives its
encoder slice in two layouts (both fp16): natural [s,k] (context matmul,
contraction over S on partitions) and transposed [k,s] (score matmul,
contraction over K on partitions). The PE contracts along the partition dim,
so both layouts are required; fp16 halves DMA and streams at the same
column rate as fp32 while PSUM accumulates in fp32.

Per-core dataflow (per batch):
  main:  psum[a128, s512] += W16[k128,a128].T @ encT16[k128,s512]   (4a*8k*4s MMs)
  tanh:  ACT Tanh(psum + bias=t2[a,1])   -> tanh16 [a128, s2048] (fused bias!)
  vdot:  psum[1, s512]   += v16[a128,1].T @ tanh16[a128,s512]       (4a*4s MMs)
  softmax (f32, single partition row): reduce_max -> Exp(bias=-max,
         accum_out=sum) -> reciprocal -> scale
  wT:    fp16 row -> DRAM -> xbar DMA-transpose -> [128,16] (s on partitions)
  ctx:   psum[1, k512]   += wT16[s128,1].T @ enc16n[s128,k512]      (16s*2k MMs)
"""

import numpy as np

import concourse.bass as bass
import concourse.bacc as bacc
import concourse.tile as tile
import concourse.mybir as mybir
from concourse.bass_utils import run_bass_kernel_spmd

FP16 = mybir.dt.float16
FP32 = mybir.dt.float32

NCORES = 8
B = 32
BPC = B // NCORES  # 4 batches per core
S = 2048
K = 1024  # 2*EH, contraction dim of enc @ W
A = 512
KO = 1024  # context output dim (2*EH)

SB = 512  # s-block for matmul free dim
N_AT = A // 128  # 4 a-tiles
N_KC = K // 128  # 8 k-chunks
N_SB = S // SB  # 4 s-blocks
N_ST = S // 128  # 16 s-tiles


def build_nc() -> bass.Bass:
    nc = bacc.Bacc()

    enc16t = nc.dram_tensor("enc16t", [BPC, K, S], FP16, kind="ExternalInput")
    enc16n = nc.dram_tensor("enc16n", [BPC, S, K], FP16, kind="ExternalInput")
    w16 = nc.dram_tensor("w16", [K, A], FP16, kind="ExternalInput")
    u16 = nc.dram_tensor("u16", [K, A], FP16, kind="ExternalInput")
    v16 = nc.dram_tensor("v16", [128, N_AT], FP16, kind="ExternalInput")
    dht16 = nc.dram_tensor("dht16", [128, N_KC * BPC], FP16, kind="ExternalInput")

    attn = nc.dram_tensor("attn", [BPC, S], FP32, kind="ExternalOutput")
    ctxv = nc.dram_tensor("ctxv", [BPC, KO], FP32, kind="ExternalOutput")

    with tile.TileContext(nc) as tc:
        with (
            tc.tile_pool(name="wu", bufs=1) as wu,
            tc.tile_pool(name="small", bufs=1) as small,
            tc.tile_pool(name="encT", bufs=15) as encT_pool,
            tc.tile_pool(name="encn", bufs=4) as encn_pool,
            tc.tile_pool(name="tanh", bufs=16) as tanh_pool,
            tc.tile_pool(name="rows", bufs=2) as rows,
            tc.tile_pool(name="wdram", bufs=2, space="DRAM") as wdram,
            tc.tile_pool(name="pm", bufs=6, space="PSUM") as pm,
            tc.tile_pool(name="psc", bufs=2, space="PSUM") as psc,
        ):
            # ---- static prologue ----
            # DMA issue order = startup critical path: W half + first encT
            # chunk feed the first main matmul; t2 inputs (v/dht/U) have slack
            # until the first tanh, so they ride between encT chunks.
            v_sb = small.tile([128, N_AT], FP16, tag="v")
            dht_sb = small.tile([128, N_KC * BPC], FP16, tag="dht")
            one32 = small.tile([1, 1], FP32, tag="one32")
            nc.vector.memset(one32[:], 1.0)
            ones128 = small.tile([1, 128], FP32, tag="ones128")
            nc.vector.memset(ones128[:], 1.0)
            w_sb = wu.tile([128, N_KC * A], FP16, tag="w")
            u_sb = wu.tile([128, N_KC * A], FP16, tag="u")
            t2_sb = small.tile([128, N_AT * BPC], FP32, tag="t2")

            def issue_w_half(h):
                nc.sync.dma_start(
                    w_sb[:, h * 4 * A : (h + 1) * 4 * A].rearrange(
                        "p (c a) -> p c a", c=4
                    ),
                    w16[h * 512 : (h + 1) * 512, :].rearrange("(c p) a -> p c a", p=128),
                )

            def issue_small():
                nc.sync.dma_start(v_sb[:], v16[:, :])
                nc.sync.dma_start(dht_sb[:], dht16[:, :])

            def issue_u_dma():
                nc.sync.dma_start(
                    u_sb[:].rearrange("p (c a) -> p c a", c=N_KC),
                    u16[:, :].rearrange("(c p) a -> p c a", p=128),
                )

            def emit_t2():
                # t2[a, b], all batches: psum[a128,4] += U[k128,a128].T @ dhT[k128,4]
                for at in range(N_AT):
                    t2_ps = psc.tile([128, BPC], FP32, tag="psc", name=f"t2ps{at}")
                    for c in range(N_KC):
                        nc.tensor.matmul(
                            t2_ps[:],
                            u_sb[:, c * A + at * 128 : c * A + (at + 1) * 128],
                            dht_sb[:, c * BPC : (c + 1) * BPC],
                            start=(c == 0),
                            stop=(c == N_KC - 1),
                        )
                    nc.scalar.activation(
                        t2_sb[:, at * BPC : (at + 1) * BPC],
                        t2_ps[:],
                        mybir.ActivationFunctionType.Copy,
                    )

            # ---- per-batch pipeline ----
            def load_encT(b):
                tiles = []
                for c in range(N_KC):
                    t = encT_pool.tile([128, S], FP16, tag="encT", name=f"eT{b}_{c}")
                    nc.sync.dma_start(t[:], enc16t[b, c * 128 : (c + 1) * 128, :])
                    tiles.append(t)
                    if b == 0 and c == 0:
                        issue_w_half(1)
                    if b == 0 and c == 1:
                        issue_small()
                        issue_u_dma()
                return tiles

            def emit_ctx(cb, wT_sb, encn_t):
                # context: psum[1, k512] += wT[s128,1].T @ encn[s128, k512]
                ctx_sb = rows.tile([1, KO], FP32, tag="ctx", name=f"ctx{cb}")
                for kb in range(KO // SB):
                    cx_ps = psc.tile([1, SB], FP32, tag="psc", name=f"cxps{cb}_{kb}")
                    for st in range(N_ST):
                        nc.tensor.matmul(
                            cx_ps[:],
                            wT_sb[:, st : st + 1],
                            encn_t[st // 8][
                                :,
                                (st % 8) * K + kb * SB : (st % 8) * K + (kb + 1) * SB,
                            ],
                            start=(st == 0),
                            stop=(st == N_ST - 1),
                        )
                    nc.scalar.activation(
                        ctx_sb[:, kb * SB : (kb + 1) * SB],
                        cx_ps[:],
                        mybir.ActivationFunctionType.Copy,
                    )
                nc.sync.dma_start(ctxv[cb : cb + 1, :], ctx_sb[:])

            pending_ctx = None
            issue_w_half(0)
            encT_next = load_encT(0)
            for b in range(BPC):
                encT_sb = encT_next

                # main matmul (k-outer: consume encT chunks as they arrive)
                # + fused-bias tanh -> tanh16[at*N_SB + sb] : [128, SB] fp16
                tanh16 = [
                    tanh_pool.tile([128, SB], FP16, tag="tanh", name=f"th{b}_{i}")
                    for i in range(N_AT * N_SB)
                ]
                score_sb = rows.tile([1, S], FP32, tag="score")
                for at in range(N_AT):
                    mm_ps = [
                        pm.tile([128, SB], FP32, tag="pm", name=f"pm{b}_{at}_{i}")
                        for i in range(N_SB)
                    ]
                    for c in range(N_KC):
                        for sb in range(N_SB):
                            nc.tensor.matmul(
                                mm_ps[sb][:],
                                w_sb[:, c * A + at * 128 : c * A + (at + 1) * 128],
                                encT_sb[c][:, sb * SB : (sb + 1) * SB],
                                start=(c == 0),
                                stop=(c == N_KC - 1),
                            )
                    if b == 0 and at == 0:
                        emit_t2()
                    for sb in range(N_SB):
                        nc.scalar.activation(
                            tanh16[at * N_SB + sb][:],
                            mm_ps[sb][:],
                            mybir.ActivationFunctionType.Tanh,
                            bias=t2_sb[:, at * BPC + b : at * BPC + b + 1],
                        )

                # prefetch next batch's encT ahead of this batch's encn
                if b + 1 < BPC:
                    encT_next = load_encT(b + 1)
                # encn_t[tt][p, t*K + k] = encn[b, (tt*8+t)*128 + p, k]
                encn_t = []
                for tt in range(2):
                    t = encn_pool.tile([128, 8 * K], FP16, tag="encn")
                    nc.sync.dma_start(
                        t[:].rearrange("p (t k) -> p t k", t=8),
                        enc16n[b, tt * 1024 : (tt + 1) * 1024, :].rearrange(
                            "(t p) k -> p t k", p=128
                        ),
                    )
                    encn_t.append(t)

                # v-dot + per-slice exp (keeps the softmax tail short)
                last = b == BPC - 1
                if last:
                    wT_ps = psc.tile([128, N_ST], FP32, tag="psc", name="wTps")
                sums4 = rows.tile([1, N_SB], FP32, tag="sums4")
                for sb in range(N_SB):
                    sc_ps = psc.tile([1, SB], FP32, tag="psc")
                    for at in range(N_AT):
                        nc.tensor.matmul(
                            sc_ps[:],
                            v_sb[:, at : at + 1],
                            tanh16[at * N_SB + sb][:],
                            start=(at == 0),
                            stop=(at == N_AT - 1),
                        )
                    # exp straight out of PSUM, per s-block. No max-subtraction:
                    # |score| <= sum|v| ~ 18, exp stays in f32 range; weights
                    # are normalized before the fp16 cast so fp16 can't overflow.
                    nc.scalar.activation(
                        score_sb[:, sb * SB : (sb + 1) * SB],
                        sc_ps[:],
                        mybir.ActivationFunctionType.Exp,
                        accum_out=sums4[:, sb : sb + 1],
                    )
                    if last:
                        # last batch: PE is idle in the tail; transpose the
                        # (unnormalized, f32) exp slices on the PE as they
                        # appear, normalize in the psum->sbuf copy below.
                        for j in range(N_ST // N_SB):
                            st = sb * (N_ST // N_SB) + j
                            nc.tensor.transpose(
                                wT_ps[:, st : st + 1],
                                score_sb[:, st * 128 : (st + 1) * 128],
                                one32[:],
                            )

                sumexp = rows.tile([1, 1], FP32, tag="sumexp")
                nc.vector.reduce_sum(sumexp[:], sums4[:], axis=mybir.AxisListType.X)
                recip = rows.tile([1, 1], FP32, tag="recip")
                nc.vector.reciprocal(recip[:], sumexp[:])
                wT_sb = rows.tile([128, N_ST], FP16, tag="wT")
                if not last:
                    # normalized fp16 weights row first (critical path to ctx)
                    w16row = rows.tile([1, S], FP16, tag="w16row")
                    nc.vector.tensor_scalar_mul(w16row[:], score_sb[:], recip[:])
                    # fp16 row -> DRAM -> xbar transpose -> [128, 16]
                    wd = wdram.tile([S], FP16, tag="wd")
                    nc.sync.dma_start(wd[:], w16row[:])
                    nc.sync.dma_start(
                        wT_sb[:],
                        wd[:].rearrange("(a b) -> a b", b=128),
                        transpose=True,
                    )
                    # f32 normalized weights -> attention output
                    nc.vector.tensor_scalar_mul(score_sb[:], score_sb[:], recip[:])
                    nc.sync.dma_start(attn[b : b + 1, :], score_sb[:])
                else:
                    # normalize + cast the already-transposed exp column block;
                    # recip must be per-partition [128,1]: broadcast it with a
                    # tiny K=1 ones-matmul
                    rb_ps = psc.tile([128, 1], FP32, tag="psc", name="rbps")
                    nc.tensor.matmul(rb_ps[:], ones128[:], recip[:])
                    recip128 = rows.tile([128, 1], FP32, tag="recip128")
                    nc.scalar.activation(
                        recip128[:], rb_ps[:], mybir.ActivationFunctionType.Copy
                    )
                    nc.vector.tensor_scalar_mul(wT_sb[:], wT_ps[:], recip128[:])
                    nc.vector.tensor_scalar_mul(score_sb[:], score_sb[:], recip[:])
                    nc.sync.dma_start(attn[b : b + 1, :], score_sb[:])

                # context is emitted one batch behind so its matmuls (which
                # wait on the softmax+transpose chain) don't block the next
                # batch's main matmuls in the PE queue.
                if pending_ctx is not None:
                    emit_ctx(*pending_ctx)
                pending_ctx = (b, wT_sb, encn_t)

            emit_ctx(*pending_ctx)

    nc.compile()
    return nc


_NC = None


def _get_nc():
    global _NC
    if _NC is None:
        _NC = build_nc()
    return _NC


def shard_inputs(decoder_hidden, encoder_outputs, W, U, v):
    decoder_hidden = np.asarray(decoder_hidden, dtype=np.float32)
    encoder_outputs = np.asarray(encoder_outputs, dtype=np.float32)
    W = np.asarray(W, dtype=np.float32)
    U = np.asarray(U, dtype=np.float32)
    v = np.asarray(v, dtype=np.float32)

    dh = np.transpose(decoder_hidden, (1, 0, 2)).reshape(B, -1)  # (B, 1024)
    w16 = W.astype(np.float16)
    u16 = U.astype(np.float16)
    v16 = np.ascontiguousarray(v.reshape(N_AT, 128).T.astype(np.float16))  # (128, 4)

    in_maps = []
    for i in range(NCORES):
        sl = slice(i * BPC, (i + 1) * BPC)
        enc16n = encoder_outputs[sl].astype(np.float16)  # (4, S, K)
        enc16t = np.ascontiguousarray(np.transpose(enc16n, (0, 2, 1)))  # (4, K, S)
        dhc = dh[sl]  # (4, 1024)
        # dht[p, c*BPC + b] = dh[b, c*128 + p]
        dht = np.ascontiguousarray(
            dhc.T.reshape(N_KC, 128, BPC).transpose(1, 0, 2).reshape(128, N_KC * BPC)
        ).astype(np.float16)
        in_maps.append(
            {
                "enc16t": enc16t,
                "enc16n": enc16n,
                "w16": w16,
                "u16": u16,
                "v16": v16,
                "dht16": dht,
            }
        )
    return in_maps


def kernel(decoder_hidden, encoder_outputs, W, U, v):
    nc = _get_nc()
    in_maps = shard_inputs(decoder_hidden, encoder_outputs, W, U, v)
    res = run_bass_kernel_spmd(nc, in_maps, list(range(NCORES)))
    attn = np.concatenate([r["attn"] for r in res.results], axis=0)  # (B, S)
    ctx = np.concatenate([r["ctxv"] for r in res.results], axis=0)  # (B, KO)
    return ctx[:, None, :].astype(np.float32), attn.astype(np.float32)


# revision 37
# speedup vs baseline: 1.1316x; 1.1316x over previous
"""Bahdanau attention kernel for 8x Trainium2 NeuronCores (Bass/Tile).

Problem (per reference):
  B=32, S=2048, EH=DH=512, A=512
  dh    = transpose(decoder_hidden,(1,0,2)).reshape(B, 1024)
  temp1 = enc @ W                  # (B,S,A)
  temp2 = dh @ U                   # (B,A)
  score = tanh(temp1 + temp2) @ v  # (B,S)
  attn  = softmax(score, -1)       # (B,S)
  ctx   = attn @ enc               # (B,1,1024)

Sharding: data-parallel over batch, 4 batches/core. Each core receives its
encoder slice in two layouts (both fp16): natural [s,k] (context matmul,
contraction over S on partitions) and transposed [k,s] (score matmul,
contraction over K on partitions). The PE contracts along the partition dim,
so both layouts are required; fp16 halves DMA and streams at the same
column rate as fp32 while PSUM accumulates in fp32.

Per-core dataflow (per batch):
  main:  psum[a128, s512] += W16[k128,a128].T @ encT16[k128,s512]   (4a*8k*4s MMs)
  tanh:  ACT Tanh(psum + bias=t2[a,1])   -> tanh16 [a128, s2048] (fused bias!)
  vdot:  psum[1, s512]   += v16[a128,1].T @ tanh16[a128,s512]       (4a*4s MMs)
  softmax (f32, single partition row): reduce_max -> Exp(bias=-max,
         accum_out=sum) -> reciprocal -> scale
  wT:    fp16 row -> DRAM -> xbar DMA-transpose -> [128,16] (s on partitions)
  ctx:   psum[1, k512]   += wT16[s128,1].T @ enc16n[s128,k512]      (16s*2k MMs)
"""

import numpy as np

import concourse.bass as bass
import concourse.bacc as bacc
import concourse.tile as tile
import concourse.mybir as mybir
from concourse.bass_utils import run_bass_kernel_spmd

FP16 = mybir.dt.float16
FP32 = mybir.dt.float32

NCORES = 8
B = 32
BPC = B // NCORES  # 4 batches per core
S = 2048
K = 1024  # 2*EH, contraction dim of enc @ W
A = 512
KO = 1024  # context output dim (2*EH)

SB = 512  # s-block for matmul free dim
N_AT = A // 128  # 4 a-tiles
N_KC = K // 128  # 8 k-chunks
N_SB = S // SB  # 4 s-blocks
N_ST = S // 128  # 16 s-tiles


def build_nc() -> bass.Bass:
    nc = bacc.Bacc()

    enc16t = nc.dram_tensor("enc16t", [BPC, K, S], FP16, kind="ExternalInput")
    enc16n = nc.dram_tensor("enc16n", [BPC, S, K], FP16, kind="ExternalInput")
    w16 = nc.dram_tensor("w16", [K, A], FP16, kind="ExternalInput")
    u16 = nc.dram_tensor("u16", [K, A], FP16, kind="ExternalInput")
    v16 = nc.dram_tensor("v16", [128, N_AT], FP16, kind="ExternalInput")
    dht16 = nc.dram_tensor("dht16", [128, N_KC * BPC], FP16, kind="ExternalInput")

    attn = nc.dram_tensor("attn", [BPC, S], FP32, kind="ExternalOutput")
    ctxv = nc.dram_tensor("ctxv", [BPC, KO], FP32, kind="ExternalOutput")

    with tile.TileContext(nc) as tc:
        with (
            tc.tile_pool(name="wu", bufs=1) as wu,
            tc.tile_pool(name="small", bufs=1) as small,
            tc.tile_pool(name="encT", bufs=15) as encT_pool,
            tc.tile_pool(name="encn", bufs=4) as encn_pool,
            tc.tile_pool(name="tanh", bufs=16) as tanh_pool,
            tc.tile_pool(name="rows", bufs=2) as rows,
            tc.tile_pool(name="wdram", bufs=2, space="DRAM") as wdram,
            tc.tile_pool(name="pm", bufs=6, space="PSUM") as pm,
            tc.tile_pool(name="psc", bufs=2, space="PSUM") as psc,
        ):
            # ---- static prologue ----
            # DMA issue order = startup critical path: W half + first encT
            # chunk feed the first main matmul; t2 inputs (v/dht/U) have slack
            # until the first tanh, so they ride between encT chunks.
            v_sb = small.tile([128, N_AT], FP16, tag="v")
            dht_sb = small.tile([128, N_KC * BPC], FP16, tag="dht")
            one32 = small.tile([1, 1], FP32, tag="one32")
            nc.vector.memset(one32[:], 1.0)
            ones128 = small.tile([1, 128], FP32, tag="ones128")
            nc.vector.memset(ones128[:], 1.0)
            w_sb = wu.tile([128, N_KC * A], FP16, tag="w")
            u_sb = wu.tile([128, N_KC * A], FP16, tag="u")
            t2_sb = small.tile([128, N_AT * BPC], FP32, tag="t2")

            def issue_w_half(h):
                nc.sync.dma_start(
                    w_sb[:, h * 4 * A : (h + 1) * 4 * A].rearrange(
                        "p (c a) -> p c a", c=4
                    ),
                    w16[h * 512 : (h + 1) * 512, :].rearrange("(c p) a -> p c a", p=128),
                )

            def issue_small():
                nc.sync.dma_start(v_sb[:], v16[:, :])
                nc.sync.dma_start(dht_sb[:], dht16[:, :])

            def issue_u_dma():
                nc.sync.dma_start(
                    u_sb[:].rearrange("p (c a) -> p c a", c=N_KC),
                    u16[:, :].rearrange("(c p) a -> p c a", p=128),
                )

            def emit_t2():
                # t2[a, b], all batches: psum[a128,4] += U[k128,a128].T @ dhT[k128,4]
                for at in range(N_AT):
                    t2_ps = psc.tile([128, BPC], FP32, tag="psc", name=f"t2ps{at}")
                    for c in range(N_KC):
                        nc.tensor.matmul(
                            t2_ps[:],
                            u_sb[:, c * A + at * 128 : c * A + (at + 1) * 128],
                            dht_sb[:, c * BPC : (c + 1) * BPC],
                            start=(c == 0),
                            stop=(c == N_KC - 1),
                        )
                    nc.scalar.activation(
                        t2_sb[:, at * BPC : (at + 1) * BPC],
                        t2_ps[:],
                        mybir.ActivationFunctionType.Copy,
                    )

            # ---- per-batch pipeline ----
            def load_encT(b):
                tiles = []
                for c in range(N_KC):
                    t = encT_pool.tile([128, S], FP16, tag="encT", name=f"eT{b}_{c}")
                    nc.sync.dma_start(t[:], enc16t[b, c * 128 : (c + 1) * 128, :])
                    tiles.append(t)
                    if b == 0 and c == 0:
                        issue_w_half(1)
                    if b == 0 and c == 1:
                        issue_small()
                        issue_u_dma()
                return tiles

            def emit_ctx(cb, wT_sb, encn_t):
                # context: psum[1, k512] += wT[s128,1].T @ encn[s128, k512]
                ctx_sb = rows.tile([1, KO], FP32, tag="ctx", name=f"ctx{cb}")
                # the two k-halves are independent M=1 chains: col-pack them
                # into column groups 0 and 64 so their streams run concurrently
                cx_ps = psc.tile([128, SB], FP32, tag="psc", name=f"cxps{cb}")
                for st in range(N_ST):
                    for kb in range(KO // SB):
                        nc.tensor.matmul(
                            cx_ps[64 * kb : 64 * kb + 1, :],
                            wT_sb[:, st : st + 1],
                            encn_t[st // 8][
                                :,
                                (st % 8) * K + kb * SB : (st % 8) * K + (kb + 1) * SB,
                            ],
                            start=(st == 0),
                            stop=(st == N_ST - 1),
                            tile_position=(0, 64 * kb),
                        )
                for kb in range(KO // SB):
                    nc.scalar.activation(
                        ctx_sb[:, kb * SB : (kb + 1) * SB],
                        cx_ps[64 * kb : 64 * kb + 1, :],
                        mybir.ActivationFunctionType.Copy,
                    )
                nc.sync.dma_start(ctxv[cb : cb + 1, :], ctx_sb[:])

            pending_ctx = None
            issue_w_half(0)
            encT_next = load_encT(0)
            for b in range(BPC):
                encT_sb = encT_next

                # main matmul (k-outer: consume encT chunks as they arrive)
                # + fused-bias tanh -> tanh16[at*N_SB + sb] : [128, SB] fp16
                tanh16 = [
                    tanh_pool.tile([128, SB], FP16, tag="tanh", name=f"th{b}_{i}")
                    for i in range(N_AT * N_SB)
                ]
                score_sb = rows.tile([1, S], FP32, tag="score")
                for at in range(N_AT):
                    mm_ps = [
                        pm.tile([128, SB], FP32, tag="pm", name=f"pm{b}_{at}_{i}")
                        for i in range(N_SB)
                    ]
                    for c in range(N_KC):
                        for sb in range(N_SB):
                            nc.tensor.matmul(
                                mm_ps[sb][:],
                                w_sb[:, c * A + at * 128 : c * A + (at + 1) * 128],
                                encT_sb[c][:, sb * SB : (sb + 1) * SB],
                                start=(c == 0),
                                stop=(c == N_KC - 1),
                            )
                    if b == 0 and at == 0:
                        emit_t2()
                    for sb in range(N_SB):
                        nc.scalar.activation(
                            tanh16[at * N_SB + sb][:],
                            mm_ps[sb][:],
                            mybir.ActivationFunctionType.Tanh,
                            bias=t2_sb[:, at * BPC + b : at * BPC + b + 1],
                        )

                # prefetch next batch's encT ahead of this batch's encn
                if b + 1 < BPC:
                    encT_next = load_encT(b + 1)
                # encn_t[tt][p, t*K + k] = encn[b, (tt*8+t)*128 + p, k]
                encn_t = []
                for tt in range(2):
                    t = encn_pool.tile([128, 8 * K], FP16, tag="encn")
                    nc.sync.dma_start(
                        t[:].rearrange("p (t k) -> p t k", t=8),
                        enc16n[b, tt * 1024 : (tt + 1) * 1024, :].rearrange(
                            "(t p) k -> p t k", p=128
                        ),
                    )
                    encn_t.append(t)

                # v-dot + per-slice exp (keeps the softmax tail short)
                last = b == BPC - 1
                if last:
                    wT_ps = pm.tile([128, N_ST], FP32, tag="pm", name="wTps")
                sums4 = rows.tile([1, N_SB], FP32, tag="sums4")
                # v-dot: 4 independent M=1 chains; col-pack pairs into column
                # groups 0/64 (two psum banks) so streams run 2x concurrent
                sc_ps2 = [
                    psc.tile([128, SB], FP32, tag="psc", name=f"scp{b}_{i}")
                    for i in range(2)
                ]
                for at in range(N_AT):
                    for sb in range(N_SB):
                        nc.tensor.matmul(
                            sc_ps2[sb // 2][64 * (sb % 2) : 64 * (sb % 2) + 1, :],
                            v_sb[:, at : at + 1],
                            tanh16[at * N_SB + sb][:],
                            start=(at == 0),
                            stop=(at == N_AT - 1),
                            tile_position=(0, 64 * (sb % 2)),
                        )
                for sb in range(N_SB):
                    # exp straight out of PSUM, per s-block. No max-subtraction:
                    # |score| <= sum|v| ~ 18, exp stays in f32 range; weights
                    # are normalized before the fp16 cast so fp16 can't overflow.
                    nc.scalar.activation(
                        score_sb[:, sb * SB : (sb + 1) * SB],
                        sc_ps2[sb // 2][64 * (sb % 2) : 64 * (sb % 2) + 1, :],
                        mybir.ActivationFunctionType.Exp,
                        accum_out=sums4[:, sb : sb + 1],
                    )
                    if last:
                        # last batch: PE is idle in the tail; transpose the
                        # (unnormalized, f32) exp slices on the PE as they
                        # appear, normalize in the psum->sbuf copy below.
                        for j in range(N_ST // N_SB):
                            st = sb * (N_ST // N_SB) + j
                            nc.tensor.transpose(
                                wT_ps[:, st : st + 1],
                                score_sb[:, st * 128 : (st + 1) * 128],
                                one32[:],
                            )

                sumexp = rows.tile([1, 1], FP32, tag="sumexp")
                nc.vector.reduce_sum(sumexp[:], sums4[:], axis=mybir.AxisListType.X)
                recip = rows.tile([1, 1], FP32, tag="recip")
                nc.vector.reciprocal(recip[:], sumexp[:])
                wT_sb = rows.tile([128, N_ST], FP16, tag="wT")
                if not last:
                    # normalized fp16 weights row first (critical path to ctx)
                    w16row = rows.tile([1, S], FP16, tag="w16row")
                    nc.vector.tensor_scalar_mul(w16row[:], score_sb[:], recip[:])
                    # fp16 row -> DRAM -> xbar transpose -> [128, 16]
                    wd = wdram.tile([S], FP16, tag="wd")
                    nc.sync.dma_start(wd[:], w16row[:])
                    nc.sync.dma_start(
                        wT_sb[:],
                        wd[:].rearrange("(a b) -> a b", b=128),
                        transpose=True,
                    )
                    # f32 normalized weights -> attention output
                    nc.vector.tensor_scalar_mul(score_sb[:], score_sb[:], recip[:])
                    nc.sync.dma_start(attn[b : b + 1, :], score_sb[:])
                else:
                    # normalize + cast the already-transposed exp column block;
                    # recip must be per-partition [128,1]: broadcast it with a
                    # tiny K=1 ones-matmul
                    rb_ps = psc.tile([128, 1], FP32, tag="psc", name="rbps")
                    nc.tensor.matmul(rb_ps[:], ones128[:], recip[:])
                    recip128 = rows.tile([128, 1], FP32, tag="recip128")
                    nc.scalar.activation(
                        recip128[:], rb_ps[:], mybir.ActivationFunctionType.Copy
                    )
                    nc.vector.tensor_scalar_mul(wT_sb[:], wT_ps[:], recip128[:])
                    nc.vector.tensor_scalar_mul(score_sb[:], score_sb[:], recip[:])
                    nc.sync.dma_start(attn[b : b + 1, :], score_sb[:])

                # context is emitted one batch behind so its matmuls (which
                # wait on the softmax+transpose chain) don't block the next
                # batch's main matmuls in the PE queue.
                if pending_ctx is not None:
                    emit_ctx(*pending_ctx)
                pending_ctx = (b, wT_sb, encn_t)

            emit_ctx(*pending_ctx)

    nc.compile()
    return nc


_NC = None


def _get_nc():
    global _NC
    if _NC is None:
        _NC = build_nc()
    return _NC


def shard_inputs(decoder_hidden, encoder_outputs, W, U, v):
    decoder_hidden = np.asarray(decoder_hidden, dtype=np.float32)
    encoder_outputs = np.asarray(encoder_outputs, dtype=np.float32)
    W = np.asarray(W, dtype=np.float32)
    U = np.asarray(U, dtype=np.float32)
    v = np.asarray(v, dtype=np.float32)

    dh = np.transpose(decoder_hidden, (1, 0, 2)).reshape(B, -1)  # (B, 1024)
    w16 = W.astype(np.float16)
    u16 = U.astype(np.float16)
    v16 = np.ascontiguousarray(v.reshape(N_AT, 128).T.astype(np.float16))  # (128, 4)

    in_maps = []
    for i in range(NCORES):
        sl = slice(i * BPC, (i + 1) * BPC)
        enc16n = encoder_outputs[sl].astype(np.float16)  # (4, S, K)
        enc16t = np.ascontiguousarray(np.transpose(enc16n, (0, 2, 1)))  # (4, K, S)
        dhc = dh[sl]  # (4, 1024)
        # dht[p, c*BPC + b] = dh[b, c*128 + p]
        dht = np.ascontiguousarray(
            dhc.T.reshape(N_KC, 128, BPC).transpose(1, 0, 2).reshape(128, N_KC * BPC)
        ).astype(np.float16)
        in_maps.append(
            {
                "enc16t": enc16t,
                "enc16n": enc16n,
                "w16": w16,
                "u16": u16,
                "v16": v16,
                "dht16": dht,
            }
        )
    return in_maps


def kernel(decoder_hidden, encoder_outputs, W, U, v):
    nc = _get_nc()
    in_maps = shard_inputs(decoder_hidden, encoder_outputs, W, U, v)
    res = run_bass_kernel_spmd(nc, in_maps, list(range(NCORES)))
    attn = np.concatenate([r["attn"] for r in res.results], axis=0)  # (B, S)
    ctx = np.concatenate([r["ctxv"] for r in res.results], axis=0)  # (B, KO)
    return ctx[:, None, :].astype(np.float32), attn.astype(np.float32)


# revision 39
# speedup vs baseline: 1.1578x; 1.0231x over previous
"""Bahdanau attention kernel for 8x Trainium2 NeuronCores (Bass/Tile).

Problem (per reference):
  B=32, S=2048, EH=DH=512, A=512
  dh    = transpose(decoder_hidden,(1,0,2)).reshape(B, 1024)
  temp1 = enc @ W                  # (B,S,A)
  temp2 = dh @ U                   # (B,A)
  score = tanh(temp1 + temp2) @ v  # (B,S)
  attn  = softmax(score, -1)       # (B,S)
  ctx   = attn @ enc               # (B,1,1024)

Sharding: data-parallel over batch, 4 batches/core. Each core receives its
encoder slice in two layouts (both fp16): natural [s,k] (context matmul,
contraction over S on partitions) and transposed [k,s] (score matmul,
contraction over K on partitions). The PE contracts along the partition dim,
so both layouts are required; fp16 halves DMA and streams at the same
column rate as fp32 while PSUM accumulates in fp32.

Per-core dataflow (per batch):
  main:  psum[a128, s512] += W16[k128,a128].T @ encT16[k128,s512]   (4a*8k*4s MMs)
  tanh:  ACT Tanh(psum + bias=t2[a,1])   -> tanh16 [a128, s2048] (fused bias!)
  vdot:  psum[1, s512]   += v16[a128,1].T @ tanh16[a128,s512]       (4a*4s MMs)
  softmax (f32, single partition row): reduce_max -> Exp(bias=-max,
         accum_out=sum) -> reciprocal -> scale
  wT:    fp16 row -> DRAM -> xbar DMA-transpose -> [128,16] (s on partitions)
  ctx:   psum[1, k512]   += wT16[s128,1].T @ enc16n[s128,k512]      (16s*2k MMs)
"""

import numpy as np

import concourse.bass as bass
import concourse.bacc as bacc
import concourse.tile as tile
import concourse.mybir as mybir
from concourse.bass_utils import run_bass_kernel_spmd

FP16 = mybir.dt.float16
FP32 = mybir.dt.float32

NCORES = 8
B = 32
BPC = B // NCORES  # 4 batches per core
S = 2048
K = 1024  # 2*EH, contraction dim of enc @ W
A = 512
KO = 1024  # context output dim (2*EH)

SB = 512  # s-block for matmul free dim
N_AT = A // 128  # 4 a-tiles
N_KC = K // 128  # 8 k-chunks
N_SB = S // SB  # 4 s-blocks
N_ST = S // 128  # 16 s-tiles


def build_nc() -> bass.Bass:
    nc = bacc.Bacc()

    enc16t = nc.dram_tensor("enc16t", [BPC, K, S], FP16, kind="ExternalInput")
    enc16n = nc.dram_tensor("enc16n", [BPC, S, K], FP16, kind="ExternalInput")
    w16 = nc.dram_tensor("w16", [K, A], FP16, kind="ExternalInput")
    u16 = nc.dram_tensor("u16", [K, A], FP16, kind="ExternalInput")
    v16 = nc.dram_tensor("v16", [128, N_AT], FP16, kind="ExternalInput")
    dht16 = nc.dram_tensor("dht16", [128, N_KC * BPC], FP16, kind="ExternalInput")

    attn = nc.dram_tensor("attn", [BPC, S], FP32, kind="ExternalOutput")
    ctxv = nc.dram_tensor("ctxv", [BPC, KO], FP32, kind="ExternalOutput")

    with tile.TileContext(nc) as tc:
        with (
            tc.tile_pool(name="wu", bufs=1) as wu,
            tc.tile_pool(name="small", bufs=1) as small,
            tc.tile_pool(name="encT", bufs=15) as encT_pool,
            tc.tile_pool(name="encn", bufs=4) as encn_pool,
            tc.tile_pool(name="tanh", bufs=16) as tanh_pool,
            tc.tile_pool(name="rows", bufs=2) as rows,
            tc.tile_pool(name="wdram", bufs=2, space="DRAM") as wdram,
            tc.tile_pool(name="pm", bufs=6, space="PSUM") as pm,
            tc.tile_pool(name="psc", bufs=2, space="PSUM") as psc,
        ):
            # ---- static prologue ----
            # DMA issue order = startup critical path: W half + first encT
            # chunk feed the first main matmul; t2 inputs (v/dht/U) have slack
            # until the first tanh, so they ride between encT chunks.
            v_sb = small.tile([128, N_AT], FP16, tag="v")
            dht_sb = small.tile([128, N_KC * BPC], FP16, tag="dht")
            one32 = small.tile([1, 1], FP32, tag="one32")
            nc.vector.memset(one32[:], 1.0)
            ones128 = small.tile([1, 128], FP32, tag="ones128")
            nc.vector.memset(ones128[:], 1.0)
            w_sb = wu.tile([128, N_KC * A], FP16, tag="w")
            u_sb = wu.tile([128, N_KC * A], FP16, tag="u")
            t2_sb = small.tile([128, N_AT * BPC], FP32, tag="t2")

            def issue_w_half(h):
                nc.sync.dma_start(
                    w_sb[:, h * 4 * A : (h + 1) * 4 * A].rearrange(
                        "p (c a) -> p c a", c=4
                    ),
                    w16[h * 512 : (h + 1) * 512, :].rearrange("(c p) a -> p c a", p=128),
                )

            def issue_small():
                nc.sync.dma_start(v_sb[:], v16[:, :])
                nc.sync.dma_start(dht_sb[:], dht16[:, :])

            def issue_u_dma():
                nc.sync.dma_start(
                    u_sb[:].rearrange("p (c a) -> p c a", c=N_KC),
                    u16[:, :].rearrange("(c p) a -> p c a", p=128),
                )

            def emit_t2():
                # t2[a, b], all batches: psum[a128,4] += U[k128,a128].T @ dhT[k128,4]
                for at in range(N_AT):
                    t2_ps = psc.tile([128, BPC], FP32, tag="psc", name=f"t2ps{at}")
                    for c in range(N_KC):
                        nc.tensor.matmul(
                            t2_ps[:],
                            u_sb[:, c * A + at * 128 : c * A + (at + 1) * 128],
                            dht_sb[:, c * BPC : (c + 1) * BPC],
                            start=(c == 0),
                            stop=(c == N_KC - 1),
                        )
                    nc.scalar.activation(
                        t2_sb[:, at * BPC : (at + 1) * BPC],
                        t2_ps[:],
                        mybir.ActivationFunctionType.Copy,
                    )

            # ---- per-batch pipeline ----
            def load_encT(b):
                tiles = []
                for c in range(N_KC):
                    t = encT_pool.tile([128, S], FP16, tag="encT", name=f"eT{b}_{c}")
                    nc.sync.dma_start(t[:], enc16t[b, c * 128 : (c + 1) * 128, :])
                    tiles.append(t)
                    if b == 0 and c == 0:
                        issue_w_half(1)
                    if b == 0 and c == 1:
                        issue_small()
                        issue_u_dma()
                return tiles

            def emit_ctx(cb, wT_sb, encn_t):
                # context: psum[1, k512] += wT[s128,1].T @ encn[s128, k512]
                ctx_sb = rows.tile([1, KO], FP32, tag="ctx", name=f"ctx{cb}")
                # the four k-quarters are independent M=1 chains: col-pack
                # them into column groups 0/32/64/96 -> 4x concurrent streams
                QN = KO // 4
                cx_ps = psc.tile([128, QN], FP32, tag="psc", name=f"cxps{cb}")
                for st in range(N_ST):
                    for q in range(4):
                        nc.tensor.matmul(
                            cx_ps[32 * q : 32 * q + 1, :],
                            wT_sb[:, st : st + 1],
                            encn_t[st // 8][
                                :, (st % 8) * K + q * QN : (st % 8) * K + (q + 1) * QN
                            ],
                            start=(st == 0),
                            stop=(st == N_ST - 1),
                            tile_position=(0, 32 * q),
                        )
                for q in range(4):
                    nc.scalar.activation(
                        ctx_sb[:, q * QN : (q + 1) * QN],
                        cx_ps[32 * q : 32 * q + 1, :],
                        mybir.ActivationFunctionType.Copy,
                    )
                nc.sync.dma_start(ctxv[cb : cb + 1, :], ctx_sb[:])

            pending_ctx = None
            issue_w_half(0)
            encT_next = load_encT(0)
            for b in range(BPC):
                encT_sb = encT_next

                # main matmul (k-outer: consume encT chunks as they arrive)
                # + fused-bias tanh -> tanh16[at*N_SB + sb] : [128, SB] fp16
                tanh16 = [
                    tanh_pool.tile([128, SB], FP16, tag="tanh", name=f"th{b}_{i}")
                    for i in range(N_AT * N_SB)
                ]
                score_sb = rows.tile([1, S], FP32, tag="score")
                for at in range(N_AT):
                    mm_ps = [
                        pm.tile([128, SB], FP32, tag="pm", name=f"pm{b}_{at}_{i}")
                        for i in range(N_SB)
                    ]
                    for c in range(N_KC):
                        for sb in range(N_SB):
                            nc.tensor.matmul(
                                mm_ps[sb][:],
                                w_sb[:, c * A + at * 128 : c * A + (at + 1) * 128],
                                encT_sb[c][:, sb * SB : (sb + 1) * SB],
                                start=(c == 0),
                                stop=(c == N_KC - 1),
                            )
                    if b == 0 and at == 0:
                        emit_t2()
                    for sb in range(N_SB):
                        nc.scalar.activation(
                            tanh16[at * N_SB + sb][:],
                            mm_ps[sb][:],
                            mybir.ActivationFunctionType.Tanh,
                            bias=t2_sb[:, at * BPC + b : at * BPC + b + 1],
                        )

                # prefetch next batch's encT ahead of this batch's encn
                if b + 1 < BPC:
                    encT_next = load_encT(b + 1)
                # encn_t[tt][p, t*K + k] = encn[b, (tt*8+t)*128 + p, k]
                encn_t = []
                for tt in range(2):
                    t = encn_pool.tile([128, 8 * K], FP16, tag="encn")
                    nc.sync.dma_start(
                        t[:].rearrange("p (t k) -> p t k", t=8),
                        enc16n[b, tt * 1024 : (tt + 1) * 1024, :].rearrange(
                            "(t p) k -> p t k", p=128
                        ),
                    )
                    encn_t.append(t)

                # v-dot + per-slice exp (keeps the softmax tail short)
                last = b == BPC - 1
                if last:
                    wT_ps = pm.tile([128, N_ST], FP32, tag="pm", name="wTps")
                sums4 = rows.tile([1, N_SB], FP32, tag="sums4")
                # v-dot: 4 independent M=1 chains; col-pack them into column
                # groups 0/32/64/96 (one psum bank) -> 4x concurrent streams
                sc_ps = psc.tile([128, SB], FP32, tag="psc", name=f"scp{b}")
                for at in range(N_AT):
                    for sb in range(N_SB):
                        nc.tensor.matmul(
                            sc_ps[32 * sb : 32 * sb + 1, :],
                            v_sb[:, at : at + 1],
                            tanh16[at * N_SB + sb][:],
                            start=(at == 0),
                            stop=(at == N_AT - 1),
                            tile_position=(0, 32 * sb),
                        )
                for sb in range(N_SB):
                    # exp straight out of PSUM, per s-block. No max-subtraction:
                    # |score| <= sum|v| ~ 18, exp stays in f32 range; weights
                    # are normalized before the fp16 cast so fp16 can't overflow.
                    nc.scalar.activation(
                        score_sb[:, sb * SB : (sb + 1) * SB],
                        sc_ps[32 * sb : 32 * sb + 1, :],
                        mybir.ActivationFunctionType.Exp,
                        accum_out=sums4[:, sb : sb + 1],
                    )
                    if last:
                        # last batch: PE is idle in the tail; transpose the
                        # (unnormalized, f32) exp slices on the PE as they
                        # appear, normalize in the psum->sbuf copy below.
                        for j in range(N_ST // N_SB):
                            st = sb * (N_ST // N_SB) + j
                            nc.tensor.transpose(
                                wT_ps[:, st : st + 1],
                                score_sb[:, st * 128 : (st + 1) * 128],
                                one32[:],
                            )

                sumexp = rows.tile([1, 1], FP32, tag="sumexp")
                nc.vector.reduce_sum(sumexp[:], sums4[:], axis=mybir.AxisListType.X)
                recip = rows.tile([1, 1], FP32, tag="recip")
                nc.vector.reciprocal(recip[:], sumexp[:])
                wT_sb = rows.tile([128, N_ST], FP16, tag="wT")
                if not last:
                    # normalized fp16 weights row first (critical path to ctx)
                    w16row = rows.tile([1, S], FP16, tag="w16row")
                    nc.vector.tensor_scalar_mul(w16row[:], score_sb[:], recip[:])
                    # fp16 row -> DRAM -> xbar transpose -> [128, 16]
                    wd = wdram.tile([S], FP16, tag="wd")
                    nc.sync.dma_start(wd[:], w16row[:])
                    nc.sync.dma_start(
                        wT_sb[:],
                        wd[:].rearrange("(a b) -> a b", b=128),
                        transpose=True,
                    )
                    # f32 normalized weights -> attention output
                    nc.vector.tensor_scalar_mul(score_sb[:], score_sb[:], recip[:])
                    nc.sync.dma_start(attn[b : b + 1, :], score_sb[:])
                else:
                    # normalize + cast the already-transposed exp column block;
                    # recip must be per-partition [128,1]: broadcast it with a
                    # tiny K=1 ones-matmul
                    rb_ps = psc.tile([128, 1], FP32, tag="psc", name="rbps")
                    nc.tensor.matmul(rb_ps[:], ones128[:], recip[:])
                    recip128 = rows.tile([128, 1], FP32, tag="recip128")
                    nc.scalar.activation(
                        recip128[:], rb_ps[:], mybir.ActivationFunctionType.Copy
                    )
                    nc.vector.tensor_scalar_mul(wT_sb[:], wT_ps[:], recip128[:])
                    nc.vector.tensor_scalar_mul(score_sb[:], score_sb[:], recip[:])
                    nc.sync.dma_start(attn[b : b + 1, :], score_sb[:])

                # context is emitted one batch behind so its matmuls (which
                # wait on the softmax+transpose chain) don't block the next
                # batch's main matmuls in the PE queue.
                if pending_ctx is not None:
                    emit_ctx(*pending_ctx)
                pending_ctx = (b, wT_sb, encn_t)

            emit_ctx(*pending_ctx)

    nc.compile()
    return nc


_NC = None


def _get_nc():
    global _NC
    if _NC is None:
        _NC = build_nc()
    return _NC


def shard_inputs(decoder_hidden, encoder_outputs, W, U, v):
    decoder_hidden = np.asarray(decoder_hidden, dtype=np.float32)
    encoder_outputs = np.asarray(encoder_outputs, dtype=np.float32)
    W = np.asarray(W, dtype=np.float32)
    U = np.asarray(U, dtype=np.float32)
    v = np.asarray(v, dtype=np.float32)

    dh = np.transpose(decoder_hidden, (1, 0, 2)).reshape(B, -1)  # (B, 1024)
    w16 = W.astype(np.float16)
    u16 = U.astype(np.float16)
    v16 = np.ascontiguousarray(v.reshape(N_AT, 128).T.astype(np.float16))  # (128, 4)

    in_maps = []
    for i in range(NCORES):
        sl = slice(i * BPC, (i + 1) * BPC)
        enc16n = encoder_outputs[sl].astype(np.float16)  # (4, S, K)
        enc16t = np.ascontiguousarray(np.transpose(enc16n, (0, 2, 1)))  # (4, K, S)
        dhc = dh[sl]  # (4, 1024)
        # dht[p, c*BPC + b] = dh[b, c*128 + p]
        dht = np.ascontiguousarray(
            dhc.T.reshape(N_KC, 128, BPC).transpose(1, 0, 2).reshape(128, N_KC * BPC)
        ).astype(np.float16)
        in_maps.append(
            {
                "enc16t": enc16t,
                "enc16n": enc16n,
                "w16": w16,
                "u16": u16,
                "v16": v16,
                "dht16": dht,
            }
        )
    return in_maps


def kernel(decoder_hidden, encoder_outputs, W, U, v):
    nc = _get_nc()
    in_maps = shard_inputs(decoder_hidden, encoder_outputs, W, U, v)
    res = run_bass_kernel_spmd(nc, in_maps, list(range(NCORES)))
    attn = np.concatenate([r["attn"] for r in res.results], axis=0)  # (B, S)
    ctx = np.concatenate([r["ctxv"] for r in res.results], axis=0)  # (B, KO)
    return ctx[:, None, :].astype(np.float32), attn.astype(np.float32)


# revision 44
# speedup vs baseline: 1.1706x; 1.0111x over previous
"""Bahdanau attention kernel for 8x Trainium2 NeuronCores (Bass/Tile).

Problem (per reference):
  B=32, S=2048, EH=DH=512, A=512
  dh    = transpose(decoder_hidden,(1,0,2)).reshape(B, 1024)
  temp1 = enc @ W                  # (B,S,A)
  temp2 = dh @ U                   # (B,A)
  score = tanh(temp1 + temp2) @ v  # (B,S)
  attn  = softmax(score, -1)       # (B,S)
  ctx   = attn @ enc               # (B,1,1024)

Sharding: data-parallel over batch, 4 batches/core. Each core receives its
encoder slice in two layouts (both fp16): natural [s,k] (context matmul,
contraction over S on partitions) and transposed [k,s] (score matmul,
contraction over K on partitions). The PE contracts along the partition dim,
so both layouts are required; fp16 halves DMA and streams at the same
column rate as fp32 while PSUM accumulates in fp32.

Per-core dataflow (per batch):
  main:  psum[a128, s512] += W16[k128,a128].T @ encT16[k128,s512]   (4a*8k*4s MMs)
  tanh:  ACT Tanh(psum + bias=t2[a,1])   -> tanh16 [a128, s2048] (fused bias!)
  vdot:  psum[1, s512]   += v16[a128,1].T @ tanh16[a128,s512]       (4a*4s MMs)
  softmax (f32, single partition row): reduce_max -> Exp(bias=-max,
         accum_out=sum) -> reciprocal -> scale
  wT:    fp16 row -> DRAM -> xbar DMA-transpose -> [128,16] (s on partitions)
  ctx:   psum[1, k512]   += wT16[s128,1].T @ enc16n[s128,k512]      (16s*2k MMs)
"""

import numpy as np

import concourse.bass as bass
import concourse.bacc as bacc
import concourse.tile as tile
import concourse.mybir as mybir
from concourse.bass_utils import run_bass_kernel_spmd

FP16 = mybir.dt.float16
FP32 = mybir.dt.float32

NCORES = 8
B = 32
BPC = B // NCORES  # 4 batches per core
S = 2048
K = 1024  # 2*EH, contraction dim of enc @ W
A = 512
KO = 1024  # context output dim (2*EH)

SB = 512  # s-block for matmul free dim
N_AT = A // 128  # 4 a-tiles
N_KC = K // 128  # 8 k-chunks
N_SB = S // SB  # 4 s-blocks
N_ST = S // 128  # 16 s-tiles


def build_nc() -> bass.Bass:
    nc = bacc.Bacc()

    enc16t = nc.dram_tensor("enc16t", [BPC, K, S], FP16, kind="ExternalInput")
    enc16n = nc.dram_tensor("enc16n", [BPC, S, K], FP16, kind="ExternalInput")
    w16 = nc.dram_tensor("w16", [K, A], FP16, kind="ExternalInput")
    u16 = nc.dram_tensor("u16", [K, A], FP16, kind="ExternalInput")
    v16 = nc.dram_tensor("v16", [128, N_AT], FP16, kind="ExternalInput")
    dht16 = nc.dram_tensor("dht16", [128, N_KC * BPC], FP16, kind="ExternalInput")

    attn = nc.dram_tensor("attn", [BPC, S], FP32, kind="ExternalOutput")
    ctxv = nc.dram_tensor("ctxv", [BPC, KO], FP32, kind="ExternalOutput")

    with tile.TileContext(nc) as tc:
        with (
            tc.tile_pool(name="wu", bufs=1) as wu,
            tc.tile_pool(name="small", bufs=1) as small,
            tc.tile_pool(name="encT", bufs=15) as encT_pool,
            tc.tile_pool(name="encn", bufs=4) as encn_pool,
            tc.tile_pool(name="tanh", bufs=16) as tanh_pool,
            tc.tile_pool(name="rows", bufs=2) as rows,
            tc.tile_pool(name="wdram", bufs=2, space="DRAM") as wdram,
            tc.tile_pool(name="pm", bufs=6, space="PSUM") as pm,
            tc.tile_pool(name="psc", bufs=2, space="PSUM") as psc,
        ):
            # ---- static prologue ----
            # DMA issue order = startup critical path: W half + first encT
            # chunk feed the first main matmul; t2 inputs (v/dht/U) have slack
            # until the first tanh, so they ride between encT chunks.
            v_sb = small.tile([128, N_AT], FP16, tag="v")
            dht_sb = small.tile([128, N_KC * BPC], FP16, tag="dht")
            one32 = small.tile([1, 1], FP32, tag="one32")
            nc.vector.memset(one32[:], 1.0)
            ones128 = small.tile([1, 128], FP32, tag="ones128")
            nc.vector.memset(ones128[:], 1.0)
            w_sb = wu.tile([128, N_KC * A], FP16, tag="w")
            u_sb = wu.tile([128, N_KC * A], FP16, tag="u")
            t2_sb = small.tile([128, N_AT * BPC], FP32, tag="t2")

            def issue_w_half(h):
                nc.sync.dma_start(
                    w_sb[:, h * 4 * A : (h + 1) * 4 * A].rearrange(
                        "p (c a) -> p c a", c=4
                    ),
                    w16[h * 512 : (h + 1) * 512, :].rearrange("(c p) a -> p c a", p=128),
                )

            def issue_small():
                nc.sync.dma_start(v_sb[:], v16[:, :])
                nc.sync.dma_start(dht_sb[:], dht16[:, :])

            def issue_u_dma():
                nc.sync.dma_start(
                    u_sb[:].rearrange("p (c a) -> p c a", c=N_KC),
                    u16[:, :].rearrange("(c p) a -> p c a", p=128),
                )

            def emit_t2():
                # t2[a, b], all batches: psum[a128,4] += U[k128,a128].T @ dhT[k128,4]
                for at in range(N_AT):
                    t2_ps = psc.tile([128, BPC], FP32, tag="psc", name=f"t2ps{at}")
                    for c in range(N_KC):
                        nc.tensor.matmul(
                            t2_ps[:],
                            u_sb[:, c * A + at * 128 : c * A + (at + 1) * 128],
                            dht_sb[:, c * BPC : (c + 1) * BPC],
                            start=(c == 0),
                            stop=(c == N_KC - 1),
                        )
                    nc.scalar.activation(
                        t2_sb[:, at * BPC : (at + 1) * BPC],
                        t2_ps[:],
                        mybir.ActivationFunctionType.Copy,
                    )

            # ---- per-batch pipeline ----
            def load_encT(b):
                tiles = []
                for c in range(N_KC):
                    t = encT_pool.tile([128, S], FP16, tag="encT", name=f"eT{b}_{c}")
                    nc.sync.dma_start(t[:], enc16t[b, c * 128 : (c + 1) * 128, :])
                    tiles.append(t)
                    if b == 0 and c == 0:
                        issue_w_half(1)
                        issue_small()
                        issue_u_dma()
                return tiles

            def emit_ctx(cb, wT_sb, encn_t):
                # context: psum[1, k512] += wT[s128,1].T @ encn[s128, k512]
                ctx_sb = rows.tile([1, KO], FP32, tag="ctx", name=f"ctx{cb}")
                # the four k-quarters are independent M=1 chains: col-pack
                # them into column groups 0/32/64/96 -> 4x concurrent streams
                QN = KO // 4
                cx_ps = psc.tile([128, QN], FP32, tag="psc", name=f"cxps{cb}")
                for st in range(N_ST):
                    for q in range(4):
                        nc.tensor.matmul(
                            cx_ps[32 * q : 32 * q + 1, :],
                            wT_sb[:, st : st + 1],
                            encn_t[st // 8][
                                :, (st % 8) * K + q * QN : (st % 8) * K + (q + 1) * QN
                            ],
                            start=(st == 0),
                            stop=(st == N_ST - 1),
                            tile_position=(0, 32 * q),
                        )
                for q in range(4):
                    nc.scalar.activation(
                        ctx_sb[:, q * QN : (q + 1) * QN],
                        cx_ps[32 * q : 32 * q + 1, :],
                        mybir.ActivationFunctionType.Copy,
                    )
                nc.sync.dma_start(ctxv[cb : cb + 1, :], ctx_sb[:])

            pending_ctx = None
            issue_w_half(0)
            encT_next = load_encT(0)
            for b in range(BPC):
                encT_sb = encT_next

                # main matmul (k-outer: consume encT chunks as they arrive)
                # + fused-bias tanh -> tanh16[at*N_SB + sb] : [128, SB] fp16
                tanh16 = [
                    tanh_pool.tile([128, SB], FP16, tag="tanh", name=f"th{b}_{i}")
                    for i in range(N_AT * N_SB)
                ]
                score_sb = rows.tile([1, S], FP32, tag="score")
                if b == 0:
                    # batch 0 is paced by encT chunk arrival: run at-PAIRS so
                    # each arriving chunk feeds 8 matmuls (saturates the PE
                    # during the load ramp). 8 psum groups: 6 from pm + 2
                    # borrowed from psc (same [128,SB] slot shape). t2 runs
                    # first (its U DMA is issued before encT chunks 1-7).
                    emit_t2()
                    for pair in range(2):
                        mm_ps = [
                            pm.tile([128, SB], FP32, tag="pm", name=f"pmA{pair}_{i}")
                            for i in range(6)
                        ] + [
                            psc.tile([128, SB], FP32, tag="psc", name=f"pmB{pair}_{i}")
                            for i in range(2)
                        ]
                        for c in range(N_KC):
                            for j in range(2 * N_SB):
                                at = pair * 2 + j // N_SB
                                sb = j % N_SB
                                nc.tensor.matmul(
                                    mm_ps[j][:],
                                    w_sb[:, c * A + at * 128 : c * A + (at + 1) * 128],
                                    encT_sb[c][:, sb * SB : (sb + 1) * SB],
                                    start=(c == 0),
                                    stop=(c == N_KC - 1),
                                )
                        for j in range(2 * N_SB):
                            at = pair * 2 + j // N_SB
                            sb = j % N_SB
                            nc.scalar.activation(
                                tanh16[at * N_SB + sb][:],
                                mm_ps[j][:],
                                mybir.ActivationFunctionType.Tanh,
                                bias=t2_sb[:, at * BPC + b : at * BPC + b + 1],
                            )
                else:
                    for at in range(N_AT):
                        mm_ps = [
                            pm.tile([128, SB], FP32, tag="pm", name=f"pm{b}_{at}_{i}")
                            for i in range(N_SB)
                        ]
                        for c in range(N_KC):
                            for sb in range(N_SB):
                                nc.tensor.matmul(
                                    mm_ps[sb][:],
                                    w_sb[:, c * A + at * 128 : c * A + (at + 1) * 128],
                                    encT_sb[c][:, sb * SB : (sb + 1) * SB],
                                    start=(c == 0),
                                    stop=(c == N_KC - 1),
                                )
                        for sb in range(N_SB):
                            nc.scalar.activation(
                                tanh16[at * N_SB + sb][:],
                                mm_ps[sb][:],
                                mybir.ActivationFunctionType.Tanh,
                                bias=t2_sb[:, at * BPC + b : at * BPC + b + 1],
                            )

                # prefetch next batch's encT ahead of this batch's encn
                if b + 1 < BPC:
                    encT_next = load_encT(b + 1)
                # encn_t[tt][p, t*K + k] = encn[b, (tt*8+t)*128 + p, k]
                encn_t = []
                for tt in range(2):
                    t = encn_pool.tile([128, 8 * K], FP16, tag="encn")
                    nc.sync.dma_start(
                        t[:].rearrange("p (t k) -> p t k", t=8),
                        enc16n[b, tt * 1024 : (tt + 1) * 1024, :].rearrange(
                            "(t p) k -> p t k", p=128
                        ),
                    )
                    encn_t.append(t)

                # v-dot + per-slice exp (keeps the softmax tail short)
                last = b == BPC - 1
                if last:
                    wT_ps = pm.tile([128, N_ST], FP32, tag="pm", name="wTps")
                sums4 = rows.tile([1, N_SB], FP32, tag="sums4")
                # v-dot: 4 independent M=1 chains; col-pack them into column
                # groups 0/32/64/96 (one psum bank) -> 4x concurrent streams
                sc_ps = psc.tile([128, SB], FP32, tag="psc", name=f"scp{b}")
                for at in range(N_AT):
                    for sb in range(N_SB):
                        nc.tensor.matmul(
                            sc_ps[32 * sb : 32 * sb + 1, :],
                            v_sb[:, at : at + 1],
                            tanh16[at * N_SB + sb][:],
                            start=(at == 0),
                            stop=(at == N_AT - 1),
                            tile_position=(0, 32 * sb),
                        )
                for sb in range(N_SB):
                    # exp straight out of PSUM, per s-block. No max-subtraction:
                    # |score| <= sum|v| ~ 18, exp stays in f32 range; weights
                    # are normalized before the fp16 cast so fp16 can't overflow.
                    nc.scalar.activation(
                        score_sb[:, sb * SB : (sb + 1) * SB],
                        sc_ps[32 * sb : 32 * sb + 1, :],
                        mybir.ActivationFunctionType.Exp,
                        accum_out=sums4[:, sb : sb + 1],
                    )
                    if last:
                        # last batch: PE is idle in the tail; transpose the
                        # (unnormalized, f32) exp slices on the PE as they
                        # appear, normalize in the psum->sbuf copy below.
                        for j in range(N_ST // N_SB):
                            st = sb * (N_ST // N_SB) + j
                            nc.tensor.transpose(
                                wT_ps[:, st : st + 1],
                                score_sb[:, st * 128 : (st + 1) * 128],
                                one32[:],
                            )

                sumexp = rows.tile([1, 1], FP32, tag="sumexp")
                nc.vector.reduce_sum(sumexp[:], sums4[:], axis=mybir.AxisListType.X)
                recip = rows.tile([1, 1], FP32, tag="recip")
                nc.vector.reciprocal(recip[:], sumexp[:])
                wT_sb = rows.tile([128, N_ST], FP16, tag="wT")
                if not last:
                    # normalized fp16 weights row first (critical path to ctx)
                    w16row = rows.tile([1, S], FP16, tag="w16row")
                    nc.vector.tensor_scalar_mul(w16row[:], score_sb[:], recip[:])
                    # fp16 row -> DRAM -> xbar transpose -> [128, 16]
                    wd = wdram.tile([S], FP16, tag="wd")
                    nc.sync.dma_start(wd[:], w16row[:])
                    nc.sync.dma_start(
                        wT_sb[:],
                        wd[:].rearrange("(a b) -> a b", b=128),
                        transpose=True,
                    )
                    # f32 normalized weights -> attention output
                    nc.vector.tensor_scalar_mul(score_sb[:], score_sb[:], recip[:])
                    nc.sync.dma_start(attn[b : b + 1, :], score_sb[:])
                else:
                    # normalize + cast the already-transposed exp column block;
                    # recip must be per-partition [128,1]: broadcast it with a
                    # tiny K=1 ones-matmul
                    rb_ps = psc.tile([128, 1], FP32, tag="psc", name="rbps")
                    nc.tensor.matmul(rb_ps[:], ones128[:], recip[:])
                    recip128 = rows.tile([128, 1], FP32, tag="recip128")
                    nc.scalar.activation(
                        recip128[:], rb_ps[:], mybir.ActivationFunctionType.Copy
                    )
                    nc.vector.tensor_scalar_mul(wT_sb[:], wT_ps[:], recip128[:])
                    nc.vector.tensor_scalar_mul(score_sb[:], score_sb[:], recip[:])
                    nc.sync.dma_start(attn[b : b + 1, :], score_sb[:])

                # context is emitted one batch behind so its matmuls (which
                # wait on the softmax+transpose chain) don't block the next
                # batch's main matmuls in the PE queue.
                if pending_ctx is not None:
                    emit_ctx(*pending_ctx)
                pending_ctx = (b, wT_sb, encn_t)

            emit_ctx(*pending_ctx)

    nc.compile()
    return nc


_NC = None


def _get_nc():
    global _NC
    if _NC is None:
        _NC = build_nc()
    return _NC


def shard_inputs(decoder_hidden, encoder_outputs, W, U, v):
    decoder_hidden = np.asarray(decoder_hidden, dtype=np.float32)
    encoder_outputs = np.asarray(encoder_outputs, dtype=np.float32)
    W = np.asarray(W, dtype=np.float32)
    U = np.asarray(U, dtype=np.float32)
    v = np.asarray(v, dtype=np.float32)

    dh = np.transpose(decoder_hidden, (1, 0, 2)).reshape(B, -1)  # (B, 1024)
    w16 = W.astype(np.float16)
    u16 = U.astype(np.float16)
    v16 = np.ascontiguousarray(v.reshape(N_AT, 128).T.astype(np.float16))  # (128, 4)

    in_maps = []
    for i in range(NCORES):
        sl = slice(i * BPC, (i + 1) * BPC)
        enc16n = encoder_outputs[sl].astype(np.float16)  # (4, S, K)
        enc16t = np.ascontiguousarray(np.transpose(enc16n, (0, 2, 1)))  # (4, K, S)
        dhc = dh[sl]  # (4, 1024)
        # dht[p, c*BPC + b] = dh[b, c*128 + p]
        dht = np.ascontiguousarray(
            dhc.T.reshape(N_KC, 128, BPC).transpose(1, 0, 2).reshape(128, N_KC * BPC)
        ).astype(np.float16)
        in_maps.append(
            {
                "enc16t": enc16t,
                "enc16n": enc16n,
                "w16": w16,
                "u16": u16,
                "v16": v16,
                "dht16": dht,
            }
        )
    return in_maps


def kernel(decoder_hidden, encoder_outputs, W, U, v):
    nc = _get_nc()
    in_maps = shard_inputs(decoder_hidden, encoder_outputs, W, U, v)
    res = run_bass_kernel_spmd(nc, in_maps, list(range(NCORES)))
    attn = np.concatenate([r["attn"] for r in res.results], axis=0)  # (B, S)
    ctx = np.concatenate([r["ctxv"] for r in res.results], axis=0)  # (B, KO)
    return ctx[:, None, :].astype(np.float32), attn.astype(np.float32)
